# revision 4
# baseline (speedup 1.0000x reference)
"""CVQNN classifier kernel for 8 Trainium2 NeuronCores.

Math: the whole quantum circuit collapses to a batch-independent affine map
(S, d) on 128-dim phase space.  Per batch row the heavy work is
    msel' = x @ W2 + d20/2          (W2 = S[rows, :64].T, shape (64, 20))
    out_k = log1p(relu(msel'_x[k]^2 + msel'_p[k]^2 + cov_k/4 - 0.5))
i.e. a (B,64) @ (64,20) matmul + elementwise tail -> (B,10).  Memory bound.

Device layout (per core, R = 125952 rows):
  - host pre-transposes x into xt2 (128, RH=R/2): partitions 0..63 carry the
    64 features of batch half A, partitions 64..127 of batch half B.
  - per super-block: DMA [128, 1536]; 12 matmuls, each lhsT = x[128, 128j:+128]
    (stationary, K=128 over BOTH halves) x rhs = Wstack [128, 40] block-diag
    [[W2, 0], [0, W2]] -> psum [128, 480] (col 40j + 20h + k).
  - DVE/ACT tail: +d, square, pair-add, +cov, relu, ln(1+.)  -> [128, 240]
  - DMA out: per-partition 960 B contiguous.
"""

import numpy as np

import concourse.bacc as bacc
import concourse.mybir as mybir
import concourse.tile as tile
from concourse.bass_utils import run_bass_kernel_spmd

N = 64          # wires
OUT = 10        # measured wires / classes
NCORES = 8
JBLK = 12                  # matmul j-blocks per super-block
TILE_W = JBLK * 128        # 1536 xt2 columns per super-block
SB = 41                    # super-blocks per core
RH = TILE_W * SB           # per-core half rows = 62976
R = 2 * RH                 # per-core rows = 125952
B_PAD = R * NCORES         # 1007616
F32 = mybir.dt.float32


# ---------------------------------------------------------------- host math
def _bs_pass(n, start, int_params):
    i = np.arange(start, n - 1, 2)
    j = i + 1
    theta = int_params[3 * i]
    phi = int_params[3 * i + 1]
    ct, st = np.cos(theta), np.sin(theta)
    cp, sp = np.cos(phi), np.sin(phi)
    S = np.eye(2 * n)
    S[i, i] = ct
    S[i, j] = -cp * st
    S[i, n + j] = -sp * st
    S[j, i] = cp * st
    S[j, j] = ct
    S[j, n + i] = -sp * st
    S[n + i, j] = sp * st
    S[n + i, n + i] = ct
    S[n + i, n + j] = -cp * st
    S[n + j, i] = sp * st
    S[n + j, n + i] = cp * st
    S[n + j, n + j] = ct
    return S


def _layer_symplectic(n, int1, squeezes, int2):
    M = _bs_pass(n, 0, int1)
    M = _bs_pass(n, 1, int1) @ M
    c = np.concatenate([np.cos(int1[2::3]), np.ones(1)])
    s = np.concatenate([np.sin(int1[2::3]), np.zeros(1)])
    Rm = np.block([[np.diag(c), np.diag(-s)], [np.diag(s), np.diag(c)]])
    Sq = np.diag(np.concatenate([np.exp(-squeezes), np.exp(squeezes)]))
    M = Sq @ (Rm @ M)
    M = _bs_pass(n, 0, int2) @ M
    M = _bs_pass(n, 1, int2) @ M
    return M


def _affine_map(layers):
    n = N
    S = np.eye(2 * n)
    d = np.zeros(2 * n)
    for int1, sq, int2, disp in layers:
        M = _layer_symplectic(n, int1, sq, int2)
        S = M @ S
        d = M @ d
        d[:n] += 2.0 * disp
    return S, d


def _device_constants(layers):
    S, d = _affine_map(layers)
    w = np.arange(OUT)
    rows = np.concatenate([w, N + w])
    cov = S @ S.T
    cov_term = cov[w, w] + cov[N + w, N + w]            # (10,)
    W2 = S[rows, :N].T                                  # (64, 20), msel' scale
    d20 = d[rows] / 2.0                                 # (20,)
    covc = cov_term / 4.0 - 0.5                         # (10,)

    wstack = np.zeros((128, 40), np.float32)
    wstack[0:64, 0:20] = W2
    wstack[64:128, 20:40] = W2
    dconst = np.broadcast_to(np.tile(d20, 24).astype(np.float32), (128, 480))
    cconst = np.broadcast_to(np.tile(covc, 24).astype(np.float32), (128, 240))
    return wstack, np.ascontiguousarray(dconst), np.ascontiguousarray(cconst)


# ---------------------------------------------------------------- bass build
def build_nc(sb_count=SB):
    rh = TILE_W * sb_count
    nc = bacc.Bacc("TRN2", target_bir_lowering=False)
    xt = nc.dram_tensor("xt", (128, rh), F32, kind="ExternalInput")
    wst = nc.dram_tensor("wstack", (128, 40), F32, kind="ExternalInput")
    dcon = nc.dram_tensor("dconst", (128, 480), F32, kind="ExternalInput")
    ccon = nc.dram_tensor("covconst", (128, 240), F32, kind="ExternalInput")
    out = nc.dram_tensor("out", (128, sb_count * 240), F32, kind="ExternalOutput")

    Square = mybir.ActivationFunctionType.Square
    Relu = mybir.ActivationFunctionType.Relu
    Ln = mybir.ActivationFunctionType.Ln

    with tile.TileContext(nc) as tc:
        with (
            tc.tile_pool(name="const", bufs=1) as cpool,
            tc.tile_pool(name="xin", bufs=3) as xpool,
            tc.tile_pool(name="mid", bufs=2) as mpool,
            tc.tile_pool(name="ob", bufs=3) as opool,
            tc.tile_pool(name="ps", bufs=2, space="PSUM") as pspool,
        ):
            w_t = cpool.tile([128, 40], F32)
            nc.sync.dma_start(w_t[:], wst[:])
            d_t = cpool.tile([128, 480], F32)
            nc.sync.dma_start(d_t[:], dcon[:])
            c_t = cpool.tile([128, 240], F32)
            nc.sync.dma_start(c_t[:], ccon[:])

            for sb in range(sb_count):
                tin = xpool.tile([128, TILE_W], F32, tag="tin")
                nc.sync.dma_start(tin[:], xt[:, sb * TILE_W:(sb + 1) * TILE_W])

                ps = pspool.tile([128, 480], F32, tag="ps")
                for j in range(JBLK):
                    nc.tensor.matmul(
                        ps[:, 40 * j:40 * j + 40],
                        tin[:, 128 * j:128 * j + 128],
                        w_t[:],
                        start=True,
                        stop=True,
                    )

                t = mpool.tile([128, 480], F32, tag="t")
                nc.vector.tensor_add(t[:], ps[:], d_t[:])
                sq = mpool.tile([128, 480], F32, tag="sq")
                nc.scalar.activation(sq[:], t[:], Square)
                sqg = sq[:].rearrange("p (g k) -> p g k", k=20)
                s = mpool.tile([128, 240], F32, tag="s")
                sv = s[:].rearrange("p (g k) -> p g k", k=10)
                nc.vector.tensor_add(sv, sqg[:, :, 0:10], sqg[:, :, 10:20])
                v = mpool.tile([128, 240], F32, tag="v")
                nc.vector.tensor_add(v[:], s[:], c_t[:])
                r = mpool.tile([128, 240], F32, tag="r")
                nc.scalar.activation(r[:], v[:], Relu)
                o = opool.tile([128, 240], F32, tag="o")
                nc.scalar.activation(o[:], r[:], Ln, bias=1.0)

                nc.sync.dma_start(out[:, sb * 240:(sb + 1) * 240], o[:])
    nc.compile()
    return nc


# ---------------------------------------------------------------- host glue
def _make_in_maps(x_batch, wstack, dconst, cconst):
    B = x_batch.shape[0]
    xpad = np.zeros((B_PAD, N), np.float32)
    xpad[:B] = x_batch
    in_maps = []
    for c in range(NCORES):
        xc = xpad[c * R:(c + 1) * R]
        xt2 = np.ascontiguousarray(
            xc.reshape(2, RH, N).transpose(0, 2, 1).reshape(128, RH)
        )
        in_maps.append(
            {"xt": xt2, "wstack": wstack, "dconst": dconst, "covconst": cconst}
        )
    return in_maps


def _decode_out(results, B):
    full = np.empty((B_PAD, OUT), np.float32)
    for c in range(NCORES):
        O = results[c]["out"].reshape(128, SB, JBLK, 2, OUT)
        for h in range(2):
            rows = O[:, :, :, h, :].transpose(1, 2, 0, 3).reshape(RH, OUT)
            base = c * R + h * RH
            full[base:base + RH] = rows
    return full[:B]


_NC_CACHE = {}


def kernel(x_batch, int1_0, squeezes_0, int2_0, disp_0,
           int1_1, squeezes_1, int2_1, disp_1, _trace=False):
    layers = [
        (np.asarray(int1_0, np.float64), np.asarray(squeezes_0, np.float64),
         np.asarray(int2_0, np.float64), np.asarray(disp_0, np.float64)),
        (np.asarray(int1_1, np.float64), np.asarray(squeezes_1, np.float64),
         np.asarray(int2_1, np.float64), np.asarray(disp_1, np.float64)),
    ]
    wstack, dconst, cconst = _device_constants(layers)
    in_maps = _make_in_maps(np.asarray(x_batch, np.float32), wstack, dconst, cconst)

    if "nc" not in _NC_CACHE:
        _NC_CACHE["nc"] = build_nc()
    nc = _NC_CACHE["nc"]

    res = run_bass_kernel_spmd(
        nc, in_maps, core_ids=list(range(NCORES)), trace=_trace
    )
    out = _decode_out(res.results, x_batch.shape[0])
    if _trace:
        return out, res
    return out


# revision 6
# speedup vs baseline: 1.8520x; 1.8520x over previous
"""CVQNN classifier kernel for 8 Trainium2 NeuronCores.

Math: the whole quantum circuit collapses to a batch-independent affine map
(S, d) on 128-dim phase space.  Per batch row the heavy work is
    msel' = x @ W2 + d20/2          (W2 = S[rows, :64].T, shape (64, 20))
    out_k = log1p(relu(msel'_x[k]^2 + msel'_p[k]^2 + cov_k/4 - 0.5))
i.e. a (B,64) @ (64,20) matmul + elementwise tail -> (B,10).  Memory bound.

Device layout (per core, R = 125952 rows):
  - host splits x into bf16 hi/lo (x = xh + xl exactly to ~2^-17 rel) and
    packs xstack (128, R) bf16: partitions 0..63 = xh features, 64..127 =
    xl features.  Same DMA bytes as fp32 x, but the PE runs single-pass
    bf16 with FWL weight loads instead of double-pass fp32.
  - per super-block (3072 cols = 24 j-blocks): 1 DMA [128, 3072] bf16.
    One matmul per j-block, stationary = xstack_j [128, 128], moving =
    wcat [128, 40] = [[Wh, Wl], [Wh, 0]]:
      psum cols 0..19  = xh.Wh + xl.Wh   (K-sum does the hi+lo merge)
      psum cols 20..39 = xh.Wl           (correction, merged on DVE)
    (dropped xl.Wl term ~ 2^-18).  Twelve j-blocks per PSUM bank.
  - tail: r1+r2, +d, square, pair-add, +cov, relu, ln(1+.) on DVE/ACT
  - DMA out [128, 240]: per-partition 960 B contiguous.
"""

import ml_dtypes
import numpy as np

import concourse.bacc as bacc
import concourse.mybir as mybir
import concourse.tile as tile
from concourse.bass_utils import run_bass_kernel_spmd

N = 64          # wires
OUT = 10        # measured wires / classes
NCORES = 8
JBLK = 24                  # matmul j-blocks per super-block
TILE_W = JBLK * 128        # 3072 xstack cols per super-block
SB = 41                    # super-blocks per core
R = TILE_W * SB            # per-core rows = 125952
B_PAD = R * NCORES         # 1007616
F32 = mybir.dt.float32
BF16 = mybir.dt.bfloat16
NPBF16 = ml_dtypes.bfloat16


# ---------------------------------------------------------------- host math
def _bs_pass(n, start, int_params):
    i = np.arange(start, n - 1, 2)
    j = i + 1
    theta = int_params[3 * i]
    phi = int_params[3 * i + 1]
    ct, st = np.cos(theta), np.sin(theta)
    cp, sp = np.cos(phi), np.sin(phi)
    S = np.eye(2 * n)
    S[i, i] = ct
    S[i, j] = -cp * st
    S[i, n + j] = -sp * st
    S[j, i] = cp * st
    S[j, j] = ct
    S[j, n + i] = -sp * st
    S[n + i, j] = sp * st
    S[n + i, n + i] = ct
    S[n + i, n + j] = -cp * st
    S[n + j, i] = sp * st
    S[n + j, n + i] = cp * st
    S[n + j, n + j] = ct
    return S


def _layer_symplectic(n, int1, squeezes, int2):
    M = _bs_pass(n, 0, int1)
    M = _bs_pass(n, 1, int1) @ M
    c = np.concatenate([np.cos(int1[2::3]), np.ones(1)])
    s = np.concatenate([np.sin(int1[2::3]), np.zeros(1)])
    Rm = np.block([[np.diag(c), np.diag(-s)], [np.diag(s), np.diag(c)]])
    Sq = np.diag(np.concatenate([np.exp(-squeezes), np.exp(squeezes)]))
    M = Sq @ (Rm @ M)
    M = _bs_pass(n, 0, int2) @ M
    M = _bs_pass(n, 1, int2) @ M
    return M


def _affine_map(layers):
    n = N
    S = np.eye(2 * n)
    d = np.zeros(2 * n)
    for int1, sq, int2, disp in layers:
        M = _layer_symplectic(n, int1, sq, int2)
        S = M @ S
        d = M @ d
        d[:n] += 2.0 * disp
    return S, d


def _device_constants(layers):
    S, d = _affine_map(layers)
    w = np.arange(OUT)
    rows = np.concatenate([w, N + w])
    cov = S @ S.T
    cov_term = cov[w, w] + cov[N + w, N + w]            # (10,)
    W2 = S[rows, :N].T.astype(np.float32)               # (64, 20), msel' scale
    d20 = (d[rows] / 2.0).astype(np.float32)            # (20,)
    covc = (cov_term / 4.0 - 0.5).astype(np.float32)    # (10,)

    Wh = W2.astype(NPBF16)
    Wl = (W2 - Wh.astype(np.float32)).astype(NPBF16)
    wcat = np.zeros((128, 40), NPBF16)                  # [[Wh, Wl], [Wh, 0]]
    wcat[0:64, 0:20] = Wh
    wcat[0:64, 20:40] = Wl
    wcat[64:128, 0:20] = Wh

    dconst = np.ascontiguousarray(
        np.broadcast_to(np.tile(d20, 24), (128, 480))).astype(np.float32)
    cconst = np.ascontiguousarray(
        np.broadcast_to(np.tile(covc, 24), (128, 240))).astype(np.float32)
    return wcat, dconst, cconst


# ---------------------------------------------------------------- bass build
def build_nc(sb_count=SB):
    rr = TILE_W * sb_count
    nc = bacc.Bacc("TRN2", target_bir_lowering=False)
    xs = nc.dram_tensor("xs", (128, rr), BF16, kind="ExternalInput")
    wst = nc.dram_tensor("wcat", (128, 40), BF16, kind="ExternalInput")
    dcon = nc.dram_tensor("dconst", (128, 480), F32, kind="ExternalInput")
    ccon = nc.dram_tensor("covconst", (128, 240), F32, kind="ExternalInput")
    out = nc.dram_tensor("out", (128, sb_count * 240), F32, kind="ExternalOutput")

    Square = mybir.ActivationFunctionType.Square
    Relu = mybir.ActivationFunctionType.Relu
    Ln = mybir.ActivationFunctionType.Ln
    Copy = mybir.ActivationFunctionType.Copy

    with tile.TileContext(nc) as tc:
        with (
            tc.tile_pool(name="const", bufs=1) as cpool,
            tc.tile_pool(name="xin", bufs=3) as xpool,
            tc.tile_pool(name="mid", bufs=2) as mpool,
            tc.tile_pool(name="ob", bufs=3) as opool,
            tc.tile_pool(name="ps", bufs=4, space="PSUM") as pspool,
        ):
            w_t = cpool.tile([128, 40], BF16)
            nc.sync.dma_start(w_t[:], wst[:])
            d_t = cpool.tile([128, 480], F32)
            nc.sync.dma_start(d_t[:], dcon[:])
            c_t = cpool.tile([128, 240], F32)
            nc.sync.dma_start(c_t[:], ccon[:])

            for sb in range(sb_count):
                tin = xpool.tile([128, TILE_W], BF16, tag="tin")
                nc.sync.dma_start(tin[:], xs[:, sb * TILE_W:(sb + 1) * TILE_W])

                tw = mpool.tile([128, 480], F32, tag="tw")
                for t in range(2):            # two psum banks of 12 j-blocks
                    ps = pspool.tile([128, 480], F32, tag="ps")
                    for jj in range(12):
                        lo = 128 * (12 * t + jj)
                        nc.tensor.matmul(
                            ps[:, 40 * jj:40 * jj + 40],
                            tin[:, lo:lo + 128], w_t[:],
                            start=True, stop=True,
                        )
                    psv = ps[:].rearrange("p (g r k) -> p g r k", r=2, k=20)
                    cb = mpool.tile([128, 240], F32, tag="cb")
                    cbv = cb[:].rearrange("p (g k) -> p g k", k=20)
                    nc.scalar.activation(cbv, psv[:, :, 1, :], Copy)
                    twv = tw[:, 240 * t:240 * t + 240].rearrange(
                        "p (g k) -> p g k", k=20)
                    nc.vector.tensor_add(twv, psv[:, :, 0, :], cbv)

                t2 = mpool.tile([128, 480], F32, tag="t2")
                nc.vector.tensor_add(t2[:], tw[:], d_t[:])
                sq = mpool.tile([128, 480], F32, tag="sq")
                nc.scalar.activation(sq[:], t2[:], Square)
                sqg = sq[:].rearrange("p (g k) -> p g k", k=20)
                s = mpool.tile([128, 240], F32, tag="s")
                sv = s[:].rearrange("p (g k) -> p g k", k=10)
                nc.vector.tensor_add(sv, sqg[:, :, 0:10], sqg[:, :, 10:20])
                v = mpool.tile([128, 240], F32, tag="v")
                nc.vector.tensor_add(v[:], s[:], c_t[:])
                r = mpool.tile([128, 240], F32, tag="r")
                nc.scalar.activation(r[:], v[:], Relu)
                o = opool.tile([128, 240], F32, tag="o")
                nc.scalar.activation(o[:], r[:], Ln, bias=1.0)

                nc.sync.dma_start(out[:, sb * 240:(sb + 1) * 240], o[:])
    nc.compile()
    return nc


# ---------------------------------------------------------------- host glue
def _make_in_maps(x_batch, wcat, dconst, cconst):
    B = x_batch.shape[0]
    xpad = np.zeros((B_PAD, N), np.float32)
    xpad[:B] = x_batch
    xh = xpad.astype(NPBF16)
    xl = (xpad - xh.astype(np.float32)).astype(NPBF16)
    in_maps = []
    for c in range(NCORES):
        sl = slice(c * R, (c + 1) * R)
        xstk = np.empty((128, R), NPBF16)
        xstk[0:64] = xh[sl].T
        xstk[64:128] = xl[sl].T
        in_maps.append({"xs": xstk, "wcat": wcat,
                        "dconst": dconst, "covconst": cconst})
    return in_maps


def _decode_out(results, B):
    full = np.empty((B_PAD, OUT), np.float32)
    for c in range(NCORES):
        O = results[c]["out"].reshape(128, SB, JBLK, OUT)
        rows = O.transpose(1, 2, 0, 3).reshape(R, OUT)
        full[c * R:(c + 1) * R] = rows
    return full[:B]


_NC_CACHE = {}


def kernel(x_batch, int1_0, squeezes_0, int2_0, disp_0,
           int1_1, squeezes_1, int2_1, disp_1, _trace=False):
    layers = [
        (np.asarray(int1_0, np.float64), np.asarray(squeezes_0, np.float64),
         np.asarray(int2_0, np.float64), np.asarray(disp_0, np.float64)),
        (np.asarray(int1_1, np.float64), np.asarray(squeezes_1, np.float64),
         np.asarray(int2_1, np.float64), np.asarray(disp_1, np.float64)),
    ]
    wcat, dconst, cconst = _device_constants(layers)
    in_maps = _make_in_maps(np.asarray(x_batch, np.float32), wcat, dconst, cconst)

    if "nc" not in _NC_CACHE:
        _NC_CACHE["nc"] = build_nc()
    nc = _NC_CACHE["nc"]

    res = run_bass_kernel_spmd(
        nc, in_maps, core_ids=list(range(NCORES)), trace=_trace
    )
    out = _decode_out(res.results, x_batch.shape[0])
    if _trace:
        return out, res
    return out
